# revision 18
# baseline (speedup 1.0000x reference)
# Trainium2 Bass kernel for nn_TemporalGCN (LSTM -> 2x GCN -> pairwise edge MLP).
#
# Sharding: pure data-parallel over B (8 batch elements -> 8 NeuronCores).
# Each core runs an identical program on its own batch element; no collectives.
#
# Key algebraic simplification: the GCN message pass
#   messages = einsum('ij,ijn->in', adj, edge @ epW.T + epb)
# collapses to   wedge @ epW.T + rowsum(adj) x epb   with
#   wedge[i,e] = sum_j adj[i,j] * edge[i,j,e]   ([N, E] only).
# Only edge_features[:, -1] is ever used, so the big [B,W,N,N,E] tensor is
# sliced on the host and never shipped.
#
# MLP structure: each 400-column tile covers 2 j-pairs (4 j's); the per-pair
# bias C_j + b1 is folded into the K=12 edge matmul via two indicator rows,
# the W1hi @ h_i term is a second accumulating matmul, and the last two
# layers run block-diagonal (2 j's per 64 lhsT columns), so the final
# 1-feature layer emits 8 j-rows per matmul.

import numpy as np

import concourse.bass as bass
import concourse.bacc as bacc
import concourse.tile as tile
from concourse import mybir
from concourse import bass_utils

H, E, F, B, W, N = 64, 5, 6, 8, 12, 200
LN_EPS = 1e-5

F32 = mybir.dt.float32
BF16 = mybir.dt.bfloat16
NPBF = mybir.dt.np(BF16)
AF = mybir.ActivationFunctionType
ALU = mybir.AluOpType
AX = mybir.AxisListType

CHUNKS = [(0, 128), (128, 72)]      # destination-node chunks over i
NTILE = 50                           # MLP tiles, 4 j's each (2 pairs side by side)


def _tile_js(m):
    """The 4 j's (a0, a1, b0, b1) covered by MLP tile m."""
    base = 8 * (m // 2) + 2 * (m % 2)
    return base, base + 1, base + 4, base + 5


# ---------------------------------------------------------------- blob layout
class _Cols:
    def __init__(self):
        self.c = 0
        self.slabs = {}

    def add(self, name, width):
        self.slabs[name] = (self.c, width)
        self.c += width

WB = _Cols()
WB.add("lhsT_x0", 256)     # 4 slots of Wih0.T at rows 32s..32s+6
WB.add("lhsT_h0", 256)     # rows 64:128 = Whh0.T
WB.add("lhsT_l1", 256)     # rows 0:64 = Whh1p.T, rows 64:128 = Wih1p.T
WB.add("identB", 128)      # bf16 identity
WB.add("lhsT_W1hi", 128)   # blockdiag(W1hi.T, W1hi.T)
WB.add("lhsT_W2", 64)      # blockdiag(W2.T, W2.T)
WB.add("lhsT_W3", 4)       # col q rows 32q:32q+32 = w3
WB.add("W1hjT", 64)        # rows 0:64
WB.add("rhs_ep0", 64)      # rows 0:5 = epW0.T, row 5 = epb0
WB.add("rhs_ep1", 64)
WB.add("rhs_gcnW0", 64)    # rows 0:64
WB.add("rhs_gcnW1", 64)
WB.add("rhs_gcnb0", 64)    # row 0
WB.add("rhs_gcnb1", 64)
WB.add("ones_row", 200)    # row 0 = 1.0
WB.add("lhsT_I2", 64)      # [I64; I64] (cross-half pair sum)
WB.add("bias1row", 256)    # row 0 = permuted layer-1 gate bias (g rows 2x)

WF = _Cols()
WF.add("biasL", 4)         # LSTM biases (g-rows pre-doubled): l0c0, l0c1, l1c0, l1c1
WF.add("scaleL", 2)        # ACT scale for chunk1: l0 = [2;1], l1 = [1;2]
WF.add("identF", 128)      # f32 identity
WF.add("b1col", 1)         # rows 0:64 = mlp_b1
WF.add("b2x4", 1)          # rows 0:128 = tile(mlp_b2, 4)
WF.add("b3col", 1)         # rows 0:128 = mlp_b3 (sigmoid bias)
WF.add("ln_g0", 64)        # row 0 (used via partition-broadcast DMA)
WF.add("ln_b0", 64)
WF.add("ln_g1", 64)
WF.add("ln_b1", 64)


def _pack_weights(inp):
    wb = np.zeros((128, WB.c), np.float32)
    wf = np.zeros((128, WF.c), np.float32)

    def put_b(name, rows, arr):
        c0, w = WB.slabs[name]
        wb[rows, c0:c0 + np.asarray(arr).shape[1]] = np.asarray(arr, np.float32)

    def put_f(name, rows, arr):
        c0, w = WF.slabs[name]
        wf[rows, c0:c0 + np.asarray(arr).shape[1]] = np.asarray(arr, np.float32)

    gsc = np.ones((256, 1), np.float32)
    gsc[128:192] = 2.0                       # g-gate rows computed pre-scaled
    Wih0, Whh0 = inp["Wih0"] * gsc, inp["Whh0"] * gsc
    b0 = inp["bih0"] + inp["bhh0"]
    # layer1 gate permutation: chunk0 = [f, i], chunk1 = [o, g]
    perm1 = np.concatenate([np.arange(64, 128), np.arange(0, 64),
                            np.arange(192, 256), np.arange(128, 192)])
    Wih1p = (inp["Wih1"] * gsc)[perm1]
    Whh1p = (inp["Whh1"] * gsc)[perm1]
    b1p = (inp["bih1"] + inp["bhh1"])[perm1]

    x0b = np.concatenate([Wih0.T, (b0 * gsc[:, 0])[None, :]], axis=0)   # [7, 256]
    for s in range(4):
        put_b("lhsT_x0", slice(32 * s, 32 * s + 7), x0b)
    put_b("bias1row", slice(0, 1), (b1p * np.concatenate(
        [np.ones(128), np.ones(64), np.full(64, 2.0)]))[None, :])
    put_b("lhsT_h0", slice(64, 128), Whh0.T)
    put_b("lhsT_l1", slice(0, 64), Whh1p.T)
    put_b("lhsT_l1", slice(64, 128), Wih1p.T)
    put_b("identB", slice(0, 128), np.eye(128))

    W1 = inp["mlp_W1"]                      # [64, 133]
    W1hi, W1hj = W1[:, :64], W1[:, 64:128]
    bd = np.zeros((128, 128), np.float32)
    bd[0:64, 0:64] = W1hi.T
    bd[64:128, 64:128] = W1hi.T
    put_b("lhsT_W1hi", slice(0, 128), bd)
    W2 = inp["mlp_W2"]                      # [32, 64]
    w2bd = np.zeros((128, 64), np.float32)
    w2bd[0:64, 0:32] = W2.T
    w2bd[64:128, 32:64] = W2.T
    put_b("lhsT_W2", slice(0, 128), w2bd)
    w3 = np.zeros((128, 4), np.float32)
    for q in range(4):
        w3[32 * q:32 * q + 32, q] = inp["mlp_W3"][0]
    put_b("lhsT_W3", slice(0, 128), w3)
    put_b("W1hjT", slice(0, 64), W1hj.T)

    for l in range(2):
        ep = np.zeros((6, 64), np.float32)
        ep[0:5] = inp["ep_W"][l].T
        ep[5] = inp["ep_b"][l]
        put_b(f"rhs_ep{l}", slice(0, 6), ep)
        put_b(f"rhs_gcnW{l}", slice(0, 64), inp["gcn_W"][l].T)
        put_b(f"rhs_gcnb{l}", slice(0, 1), inp["gcn_b"][l][None, :])
    put_b("ones_row", slice(0, 1), np.ones((1, 200)))

    put_f("identF", slice(0, 128), np.eye(128))
    put_b("lhsT_I2", slice(0, 128), np.concatenate([np.eye(64), np.eye(64)], axis=0))
    put_f("b1col", slice(0, 64), inp["mlp_b1"][:, None])
    put_f("b2x4", slice(0, 128), np.tile(inp["mlp_b2"], 4)[:, None])
    put_f("b3col", slice(0, 128),
          np.full((128, 1), float(np.asarray(inp["mlp_b3"]).reshape(-1)[0]), np.float32))
    for l in range(2):
        put_f(f"ln_g{l}", slice(0, 1), inp["ln_g"][l][None, :])
        put_f(f"ln_b{l}", slice(0, 1), inp["ln_b"][l][None, :])

    return wb.astype(NPBF), wf


def _pack_embase(inp):
    """[12, NTILE*128] bf16: per-tile lhsT base for the K=12 mm1e.
    Rows 0:10 = blockdiag(W1e.T, W1e.T); rows 10:12 zero (Cb written on device)."""
    W1e = np.asarray(inp["mlp_W1"][:, 128:133], np.float32)  # [64, 5]
    blk = np.zeros((12, 128), np.float32)
    blk[0:5, 0:64] = W1e.T
    blk[5:10, 64:128] = W1e.T
    em = np.tile(blk, (1, NTILE))
    return em.astype(NPBF)


def _pack_core(inp, b):
    """Per-core (per-batch-element) data blobs."""
    edge = np.asarray(inp["edge_features"][b, -1], np.float32)   # [N, N, E]
    adj = np.asarray(inp["adjacency"][b], np.float32)            # [N, N]
    node = np.asarray(inp["node_features"][b], np.float32)       # [W, N, F]

    # xpad [128, 600]: tile k, slot s, rows 32s:32s+6 = node[4k+s].T
    xt = node.transpose(0, 2, 1)                                  # [W, F, N]
    xp = np.zeros((3, 4, 32, 200), np.float32)
    xp[:, :, 0:6] = xt.reshape(3, 4, 6, 200)
    xp[:, :, 6] = 1.0                        # ones row: layer-0 gate bias via K=7
    xpad_full = np.zeros((128, 600), np.float32)
    for s in range(4):
        for k in range(3):
            xpad_full[32 * s:32 * s + 32, 200 * k:200 * k + 200] = xp[k, s]

    # edge_w [128, 2000]: (i, e-major*j) layout, chunk1 in cols 1000:2000
    ew = edge.transpose(0, 2, 1).reshape(200, 1000)               # [i, e*200+j]
    edge_w = np.zeros((128, 2000), np.float32)
    edge_w[:, 0:1000] = ew[0:128]
    edge_w[0:72, 1000:2000] = ew[128:200]

    # edge_mlp [12, NTILE*400]: tile m cols 400m:400m+400 = pairs (a, b);
    # rows 0:5 = e of half-0 j, rows 5:10 = e of half-1 j, rows 10/11 =
    # indicator for pair a / b (selects the Cb lhsT rows).
    em = edge.transpose(1, 2, 0)                                  # [j, e, i]
    edge_mlp = np.zeros((12, NTILE * 400), np.float32)
    for m in range(NTILE):
        a0, a1, b0_, b1_ = _tile_js(m)
        c = 400 * m
        edge_mlp[0:5, c:c + 200] = em[a0]
        edge_mlp[5:10, c:c + 200] = em[a1]
        edge_mlp[0:5, c + 200:c + 400] = em[b0_]
        edge_mlp[5:10, c + 200:c + 400] = em[b1_]
        edge_mlp[10, c:c + 200] = 1.0
        edge_mlp[11, c + 200:c + 400] = 1.0

    return {
        "xpad": xpad_full.astype(NPBF),
        "edge_w": edge_w.astype(NPBF),
        "edge_mlp": edge_mlp.astype(NPBF),
        "adj": adj.astype(NPBF),
    }


# ---------------------------------------------------------------- bass program
def _build(debug=False):
    nc = bacc.Bacc("TRN2", target_bir_lowering=False)
    d = {}
    d["xpad"] = nc.dram_tensor("xpad", [128, 600], BF16, kind="ExternalInput").ap()
    d["edge_w"] = nc.dram_tensor("edge_w", [128, 2000], BF16, kind="ExternalInput").ap()
    d["edge_mlp"] = nc.dram_tensor("edge_mlp", [12, NTILE * 400], BF16,
                                   kind="ExternalInput").ap()
    d["embase"] = nc.dram_tensor("embase", [12, NTILE * 128], BF16,
                                 kind="ExternalInput").ap()
    d["adj"] = nc.dram_tensor("adj", [200, 200], BF16, kind="ExternalInput").ap()
    d["wb16"] = nc.dram_tensor("wb16", [128, WB.c], BF16, kind="ExternalInput").ap()
    d["wf32"] = nc.dram_tensor("wf32", [128, WF.c], F32, kind="ExternalInput").ap()
    d_out = nc.dram_tensor("outT", [200, 200], F32, kind="ExternalOutput").ap()
    d["cbst"] = nc.dram_tensor("cbst_scratch", [100, 128], BF16, kind="Internal").ap()
    dbg = {}
    if debug:
        for nm, shp, dt in [("dbg_h", [64, 200], BF16), ("dbg_h1", [64, 200], BF16),
                            ("dbg_h2", [64, 200], BF16), ("dbg_w6", [200, 6], F32),
                            ("dbg_cb", [64, 200], F32)]:
            dbg[nm] = nc.dram_tensor(nm, shp, dt, kind="ExternalOutput").ap()

    with tile.TileContext(nc) as tc:
        _body(nc, tc, d, d_out, dbg)
    nc.compile()
    return nc


def _body(nc, tc, d, d_out, dbg):
    import contextlib
    ctx = contextlib.ExitStack()
    with ctx:
        consts = ctx.enter_context(tc.tile_pool(name="consts", bufs=1))
        work = ctx.enter_context(tc.tile_pool(name="work", bufs=3))

        # ---------------- constants + inputs (LSTM-critical ones first)
        wb = consts.tile([128, WB.c], BF16)
        nc.sync.dma_start(out=wb, in_=d["wb16"])
        xpad = consts.tile([128, 600], BF16)
        nc.sync.dma_start(out=xpad, in_=d["xpad"])
        wf = consts.tile([128, WF.c], F32)
        nc.sync.dma_start(out=wf, in_=d["wf32"])

        def WBS(name, rows=slice(0, 128)):
            c0, w = WB.slabs[name]
            return wb[rows, c0:c0 + w]

        def WFS(name, rows=slice(0, 128)):
            c0, w = WF.slabs[name]
            return wf[rows, c0:c0 + w]

        edge_w = consts.tile([128, 2000], BF16)
        nc.gpsimd.dma_start(out=edge_w, in_=d["edge_w"])
        edge_mlp = consts.tile([12, NTILE * 400], BF16)
        nc.gpsimd.dma_start(out=edge_mlp, in_=d["edge_mlp"])
        lhsT_em = consts.tile([12, NTILE * 128], BF16)
        nc.gpsimd.dma_start(out=lhsT_em, in_=d["embase"])

        # adj broadcast x5 (partition rows = i, free = (e, j) with e step 0)
        adjx = []
        for k, (i0, ck) in enumerate(CHUNKS):
            t = consts.tile([128, 5, 200], BF16, tag=f"adjx{k}")
            src = bass.AP(tensor=d["adj"].tensor, offset=i0 * 200,
                          ap=[[200, ck], [0, 5], [1, 200]])
            nc.sync.dma_start(out=t[0:ck], in_=src)
            adjx.append(t)

        eps_t = consts.tile([128, 1], F32)
        nc.vector.memset(eps_t, LN_EPS)

        # PE warmup + keepalive: the HAM clock-gate opens only after a ~3.4us
        # continuously-busy window and shuts after any ~3.4us idle window, so
        # burst at the start and sprinkle filler matmuls into every gap.
        ps_warm = ctx.enter_context(tc.tile_pool(name="ps_warm", bufs=1, space="PSUM"))
        pw = ps_warm.tile([128, 512], F32)

        def pe_fill(n, cols=256):
            for _ in range(n):
                nc.tensor.matmul(pw[:, 0:cols], wb[:, 0:128], wb[:, 0:cols],
                                 start=True, stop=True, skip_group_check=True)

        pe_fill(20, 512)

        # ---------------- wedge: wedge[i, e] = sum_j adj[i,j]*edge[i,j,e]
        w6 = []
        for k, (i0, ck) in enumerate(CHUNKS):
            tmp = work.tile([128, 1000], BF16, tag="wtmp")
            nc.gpsimd.tensor_mul(
                out=tmp[0:ck],
                in0=edge_w[0:ck, 1000 * k:1000 * k + 1000],
                in1=adjx[k][0:ck].rearrange("p e j -> p (e j)"),
            )
            w6c = consts.tile([128, 6], F32, tag=f"w6_{k}")
            for e in range(5):
                nc.vector.reduce_sum(out=w6c[0:ck, e:e + 1],
                                     in_=tmp[0:ck, 200 * e:200 * e + 200], axis=AX.X)
            nc.vector.reduce_sum(out=w6c[0:ck, 5:6],
                                 in_=adjx[k][0:ck, 0:1, :], axis=AX.X)
            w6.append(w6c)
        if dbg:
            nc.gpsimd.dma_start(out=dbg["dbg_w6"][0:128, :], in_=w6[0][0:128])
            nc.gpsimd.dma_start(out=dbg["dbg_w6"][128:200, :], in_=w6[1][0:72])

        # LN scale/bias broadcast tiles (only needed by the GCN)
        lng, lnb = [], []
        for l in range(2):
            g = consts.tile([128, 64], F32, tag=f"lng{l}")
            bt = consts.tile([128, 64], F32, tag=f"lnb{l}")
            for t, nm in ((g, f"ln_g{l}"), (bt, f"ln_b{l}")):
                c0, wdt = WF.slabs[nm]
                ap_b = bass.AP(tensor=d["wf32"].tensor, offset=c0, ap=[[0, 128], [1, 64]])
                nc.gpsimd.dma_start(out=t, in_=ap_b)
            lng.append(g)
            lnb.append(bt)

        # ---------------- LSTM (12 steps x 2 layers)
        # Gate layout per layer-step: one [128, 400] psum tile G;
        #   cols 0:200   = chunk0 (sigmoid gates)
        #   cols 200:400 = chunk1 (layer0: [g; o], layer1: [o; g])
        # chunk1 is evaluated with one sigmoid whose per-partition scale is 2
        # on the g rows; tanh(g) = 2*sigmoid(2g) - 1 is fixed up on gpsimd.
        hTfin = consts.tile([128, 200], BF16, tag="hTfin")
        with tc.tile_pool(name="ps_g", bufs=4, space="PSUM") as ps_g, \
             tc.tile_pool(name="ps_c", bufs=2, space="PSUM") as ps_c, \
             tc.tile_pool(name="lstm", bufs=4) as lp:
            Y = [lp.tile([128, 200], BF16, tag="Y", name=f"Y{t}") for t in range(12)] + [hTfin]
            c_prev = None
            for t in range(12):
                ct = ps_c.tile([128, 400], F32, tag="c")
                s = t % 4
                k = t // 4
                rhs_x = xpad[32 * s:32 * s + 7, 200 * k:200 * k + 200]
                # ---- layer 0: chunk0 = [i; f], chunk1 = [g; o]
                G0 = ps_g.tile([128, 400], F32, tag="g")
                for c in range(2):
                    sl = G0[:, 200 * c:200 * c + 200]
                    nc.tensor.matmul(sl, WBS("lhsT_x0", slice(32 * s, 32 * s + 7))[:, 128 * c:128 * c + 128],
                                     rhs_x, start=True, stop=(t == 0),
                                     tile_position=(32 * s, 0), skip_group_check=True)
                    if t > 0:
                        nc.tensor.matmul(sl, WBS("lhsT_h0", slice(64, 128))[:, 128 * c:128 * c + 128],
                                         Y[t - 1][64:128, :], start=False, stop=True,
                                         skip_group_check=True)
                # one sigmoid over all 4 gates (g rows arrive pre-scaled 2x);
                # S0 cols 0:200 = [si; sf], cols 200:400 = [sig(2g); so]
                S0 = lp.tile([128, 400], BF16, tag="S")
                nc.scalar.activation(out=S0, in_=G0, func=AF.Sigmoid)
                # tanh(g) = 2*sig(2g) - 1
                nc.vector.tensor_scalar(out=S0[0:64, 200:400], in0=S0[0:64, 200:400],
                                        scalar1=2.0, scalar2=-1.0, op0=ALU.mult, op1=ALU.add)
                M0 = lp.tile([128, 200], BF16, tag="M")
                nc.vector.tensor_mul(out=M0[0:64], in0=S0[0:64, 0:200], in1=S0[0:64, 200:400])
                if t > 0:
                    nc.vector.tensor_mul(out=M0[64:128], in0=S0[64:128, 0:200],
                                         in1=c_prev[64:128, 0:200])
                else:
                    nc.vector.memset(M0[64:128], 0.0)
                nc.tensor.matmul(ct[64:128, 0:200], WBS("lhsT_I2"), M0, start=True,
                                 stop=True, skip_group_check=True)
                pe_fill(4)
                TC0 = lp.tile([128, 200], BF16, tag="TC")
                nc.scalar.activation(out=TC0[64:128], in_=ct[64:128, 0:200], func=AF.Tanh)
                nc.vector.tensor_mul(out=Y[t][64:128], in0=S0[64:128, 200:400],
                                     in1=TC0[64:128])
                # ---- layer 1: chunk0 = [f; i], chunk1 = [o; g]
                G1 = ps_g.tile([128, 400], F32, tag="g")
                for c in range(2):
                    sl = G1[:, 200 * c:200 * c + 200]
                    if t == 0:
                        nc.tensor.matmul(sl, WBS("lhsT_l1", slice(64, 128))[:, 128 * c:128 * c + 128],
                                         Y[t][64:128, :], start=True, stop=False,
                                         skip_group_check=True)
                    else:
                        nc.tensor.matmul(sl, WBS("lhsT_l1")[:, 128 * c:128 * c + 128],
                                         Y[t], start=True, stop=False, skip_group_check=True)
                    nc.tensor.matmul(sl, WBS("bias1row", slice(0, 1))[:, 128 * c:128 * c + 128],
                                     WBS("ones_row", slice(0, 1)), start=False, stop=True,
                                     skip_group_check=True)
                # S1 cols 0:200 = [sf; si], cols 200:400 = [so; sig(2g)]
                S1 = lp.tile([128, 400], BF16, tag="S")
                nc.scalar.activation(out=S1, in_=G1, func=AF.Sigmoid)
                nc.vector.tensor_scalar(out=S1[64:128, 200:400], in0=S1[64:128, 200:400],
                                        scalar1=2.0, scalar2=-1.0, op0=ALU.mult, op1=ALU.add)
                M1 = lp.tile([128, 200], BF16, tag="M")
                if t > 0:
                    nc.vector.tensor_mul(out=M1[0:64], in0=S1[0:64, 0:200],
                                         in1=c_prev[0:64, 200:400])
                else:
                    nc.vector.memset(M1[0:64], 0.0)
                nc.vector.tensor_mul(out=M1[64:128], in0=S1[64:128, 0:200],
                                     in1=S1[64:128, 200:400])
                nc.tensor.matmul(ct[0:64, 200:400], WBS("lhsT_I2"), M1, start=True,
                                 stop=True, skip_group_check=True)
                pe_fill(4)
                TC1 = lp.tile([128, 200], BF16, tag="TC")
                nc.scalar.activation(out=TC1[0:64], in_=ct[0:64, 200:400], func=AF.Tanh)
                nc.vector.tensor_mul(out=Y[t + 1][0:64], in0=S1[0:64, 200:400],
                                     in1=TC1[0:64])
                c_prev = ct
        if dbg:
            nc.gpsimd.dma_start(out=dbg["dbg_h"], in_=hTfin[0:64])

        # ---------------- wedge transpose ([i,6] chunks -> wedgeT6 [6, 200])
        wedgeT6 = consts.tile([6, 200], BF16)
        with tc.tile_pool(name="ps_wt", bufs=2, space="PSUM") as ps_wt:
            for k, (i0, ck) in enumerate(CHUNKS):
                pwt = ps_wt.tile([6, 128], F32, tag="wt")
                nc.tensor.transpose(pwt[:, 0:ck], w6[k][0:ck], WFS("identF", slice(0, ck))[:, 0:ck])
                nc.vector.tensor_copy(out=wedgeT6[:, i0:i0 + ck], in_=pwt[:, 0:ck])

        # ---------------- GCN (2 layers)
        hT_cur = hTfin
        with tc.tile_pool(name="ps_u", bufs=2, space="PSUM") as ps_u, \
             tc.tile_pool(name="ps_t", bufs=2, space="PSUM") as ps_t:
            for l in range(2):
                hT_next = consts.tile([64, 200], BF16, tag=f"hT{l + 1}")
                for k, (i0, ck) in enumerate(CHUNKS):
                    pu = ps_u.tile([128, 64], F32, tag="u")
                    nc.tensor.matmul(pu[0:ck], wedgeT6[:, i0:i0 + ck], WBS(f"rhs_ep{l}", slice(0, 6)),
                                     start=True, stop=False)
                    nc.tensor.matmul(pu[0:ck], hT_cur[0:64, i0:i0 + ck], WBS(f"rhs_gcnW{l}", slice(0, 64)),
                                     start=False, stop=False)
                    nc.tensor.matmul(pu[0:ck], WBS("ones_row", slice(0, 1))[:, i0:i0 + ck],
                                     WBS(f"rhs_gcnb{l}", slice(0, 1)), start=False, stop=True)
                    stats = work.tile([128, nc.vector.BN_STATS_DIM], F32, tag="bst")
                    nc.vector.bn_stats(out=stats[0:ck], in_=pu[0:ck])
                    mv = work.tile([128, nc.vector.BN_AGGR_DIM], F32, tag="mv")
                    nc.vector.bn_aggr(out=mv[0:ck], in_=stats[0:ck])
                    rstd = work.tile([128, 1], F32, tag="rstd")
                    nc.scalar.activation(out=rstd[0:ck], in_=mv[0:ck, 1:2], func=AF.Sqrt,
                                         bias=eps_t[0:ck])
                    nc.vector.reciprocal(out=rstd[0:ck], in_=rstd[0:ck])
                    xn = work.tile([128, 64], F32, tag="xn")
                    nc.vector.tensor_scalar(out=xn[0:ck], in0=pu[0:ck], scalar1=mv[0:ck, 0:1],
                                            scalar2=rstd[0:ck], op0=ALU.subtract, op1=ALU.mult)
                    nc.vector.tensor_mul(out=xn[0:ck], in0=xn[0:ck], in1=lng[l][0:ck])
                    nc.vector.tensor_add(out=xn[0:ck], in0=xn[0:ck], in1=lnb[l][0:ck])
                    hnew = work.tile([128, 64], BF16, tag="hnew")
                    nc.scalar.activation(out=hnew[0:ck], in_=xn[0:ck], func=AF.Relu)
                    pt = ps_t.tile([64, 128], BF16, tag="pt")
                    nc.tensor.transpose(pt[:, 0:ck], hnew[0:ck], WBS("identB", slice(0, ck))[:, 0:ck])
                    nc.vector.tensor_copy(out=hT_next[:, i0:i0 + ck], in_=pt[:, 0:ck])
                    pe_fill(8)
                hT_cur = hT_next
                if dbg:
                    nc.gpsimd.dma_start(out=dbg[f"dbg_h{l + 1}"], in_=hT_next)

        # ---------------- MLP prep: hT4, Cb -> CbS -> CbST -> lhsT_em rows 10:12
        hT4 = consts.tile([128, 400], BF16, tag="hT4")
        for r in range(2):
            for c in range(2):
                nc.sync.dma_start(out=hT4[64 * r:64 * r + 64, 200 * c:200 * c + 200],
                                    in_=hT_cur[0:64])
        Cb = consts.tile([64, 200], F32, tag="Cb")
        CbS = consts.tile([128, 100], F32, tag="CbS")
        CbST = consts.tile([100, 128], BF16, tag="CbST")
        with tc.tile_pool(name="ps_prep", bufs=2, space="PSUM") as ps_prep:
            pC = ps_prep.tile([64, 200], F32)
            nc.tensor.matmul(pC, WBS("W1hjT", slice(0, 64)), hT_cur[0:64], start=True, stop=True)
            nc.scalar.activation(out=Cb, in_=pC, func=AF.Identity, bias=WFS("b1col", slice(0, 64)))
            # CbS columns are ordered so that after transposing, rows (2m, 2m+1)
            # are exactly the two Cb lhsT rows of MLP tile m. Column c of CbS
            # holds pair perm[c]; the even-j source indices follow
            # j = 8a + 2x + 4y (+1 for the odd half), matching _tile_js.
            for half in range(2):
                half_sl = CbS[64 * half:64 * half + 64]
                for r in range(2):
                    for x in range(2):
                        srcap = bass.AP(tensor=Cb.tensor,
                                        offset=Cb.offset + half + 4 * r + 2 * x,
                                        ap=[Cb.ap[0], [8, 25]])
                        dstap = bass.AP(tensor=half_sl.tensor,
                                        offset=half_sl.offset + r + 2 * x,
                                        ap=[half_sl.ap[0], [4, 25]])
                        eng = nc.sync if x == 0 else nc.gpsimd
                        eng.dma_start(out=dstap, in_=srcap)
            pT = ps_prep.tile([100, 128], F32)
            nc.tensor.transpose(pT, CbS, WFS("identF"))
            nc.vector.tensor_copy(out=CbST, in_=pT)
        # roundtrip through DRAM: partition pairs (2m, 2m+1) -> rows 10:12 of
        # every lhsT_em block (the 2-partition-dim gather SBUF APs can't express)
        nc.sync.dma_start(out=d["cbst"], in_=CbST)
        rt = bass.AP(tensor=d["cbst"].tensor, offset=0,
                     ap=[[128, 2], [256, 50], [1, 128]])
        nc.sync.dma_start(out=lhsT_em[10:12, :].rearrange("p (m n) -> p m n", n=128),
                          in_=rt)
        if dbg:
            nc.gpsimd.dma_start(out=dbg["dbg_cb"], in_=Cb)

        pe_fill(16, 512)   # re-open the clock gate before the MLP burst

        # ---------------- MLP main loop (50 tiles of 4 j's)
        with tc.tile_pool(name="ps1", bufs=4, space="PSUM") as ps1, \
             tc.tile_pool(name="ps2", bufs=2, space="PSUM") as ps2, \
             tc.tile_pool(name="ps3", bufs=1, space="PSUM") as ps3:
            p3 = ps3.tile([128, 400], F32)
            nc.vector.memset(p3, 0.0)
            p2 = None
            stage = None
            for m in range(NTILE):
                p1 = ps1.tile([128, 400], F32, tag="p1")
                nc.tensor.matmul(p1, WBS("lhsT_W1hi"), hT4, start=True, stop=False)
                nc.tensor.matmul(p1, lhsT_em[:, 128 * m:128 * m + 128],
                                 edge_mlp[:, 400 * m:400 * m + 400],
                                 start=False, stop=True)
                z1 = work.tile([128, 400], BF16, tag="z1")
                if m % 2 == 0:
                    nc.scalar.activation(out=z1, in_=p1, func=AF.Relu)
                else:
                    nc.vector.tensor_scalar_max(out=z1, in0=p1, scalar1=0.0)
                if m % 2 == 0:
                    p2 = ps2.tile([128, 400], F32, tag="p2")
                nc.tensor.matmul(p2[64 * (m % 2):64 * (m % 2) + 64, :],
                                 WBS("lhsT_W2"), z1, start=True, stop=True,
                                 skip_group_check=True)
                if m % 2 == 1:
                    v = m // 2
                    z2r = work.tile([128, 400], BF16, tag="z2")
                    nc.vector.tensor_scalar(out=z2r, in0=p2, scalar1=WFS("b2x4"),
                                            scalar2=0.0, op0=ALU.add, op1=ALU.max)
                    q = v % 4
                    nc.tensor.matmul(p3[32 * q:32 * q + 4, :], WBS("lhsT_W3"), z2r,
                                     start=True, stop=True, skip_group_check=True,
                                     tile_position=(0, 32 * q))
                    if q == 0:
                        stage = work.tile([128, 400], F32, tag="stage")
                    if q == 3 or v == 24:
                        nc.scalar.activation(out=stage, in_=p3, func=AF.Sigmoid,
                                             bias=WFS("b3col"))
                        for q2 in range(q + 1):
                            vv = (v // 4) * 4 + q2
                            dst = bass.AP(tensor=d_out.tensor, offset=8 * vv * 200,
                                          ap=[[200, 4], [800, 2], [1, 200]])
                            nc.sync.dma_start(
                                out=dst,
                                in_=stage[32 * q2:32 * q2 + 4, :].rearrange(
                                    "p (u i) -> p u i", u=2))


# ---------------------------------------------------------------- entry points
_CACHE = {}


def _get_nc(debug=False):
    key = bool(debug)
    if key not in _CACHE:
        _CACHE[key] = _build(debug)
    return _CACHE[key]


def _make_in_maps(inputs):
    wb, wf = _pack_weights(inputs)
    emb = _pack_embase(inputs)
    maps = []
    for b in range(B):
        m = _pack_core(inputs, b)
        m["wb16"] = wb
        m["wf32"] = wf
        m["embase"] = emb
        maps.append(m)
    return maps


def _run(inputs, trace=False, debug=False):
    nc = _get_nc(debug)
    in_maps = _make_in_maps(inputs)
    res = bass_utils.run_bass_kernel_spmd(nc, in_maps, core_ids=list(range(B)), trace=trace)
    outs = np.stack([res.results[b]["outT"].T for b in range(B)], axis=0).astype(np.float32)
    return outs, res


def kernel(**inputs):
    inputs = {k: np.asarray(v) for k, v in inputs.items()}
    outs, _ = _run(inputs, trace=False)
    return outs


# revision 22
# speedup vs baseline: 1.0526x; 1.0526x over previous
# Trainium2 Bass kernel for nn_TemporalGCN (LSTM -> 2x GCN -> pairwise edge MLP).
#
# Sharding: pure data-parallel over B (8 batch elements -> 8 NeuronCores).
# Each core runs an identical program on its own batch element; no collectives.
#
# Key algebraic simplification: the GCN message pass
#   messages = einsum('ij,ijn->in', adj, edge @ epW.T + epb)
# collapses to   wedge @ epW.T + rowsum(adj) x epb   with
#   wedge[i,e] = sum_j adj[i,j] * edge[i,j,e]   ([N, E] only).
# Only edge_features[:, -1] is ever used, so the big [B,W,N,N,E] tensor is
# sliced on the host and never shipped.
#
# MLP structure: each 400-column tile covers 2 j-pairs (4 j's); the per-pair
# bias C_j + b1 is folded into the K=12 edge matmul via two indicator rows,
# the W1hi @ h_i term is a second accumulating matmul, and the last two
# layers run block-diagonal (2 j's per 64 lhsT columns), so the final
# 1-feature layer emits 8 j-rows per matmul.

import numpy as np

import concourse.bass as bass
import concourse.bacc as bacc
import concourse.tile as tile
from concourse import mybir
from concourse import bass_utils

H, E, F, B, W, N = 64, 5, 6, 8, 12, 200
LN_EPS = 1e-5

F32 = mybir.dt.float32
BF16 = mybir.dt.bfloat16
NPBF = mybir.dt.np(BF16)
AF = mybir.ActivationFunctionType
ALU = mybir.AluOpType
AX = mybir.AxisListType

CHUNKS = [(0, 128), (128, 72)]      # destination-node chunks over i
NTILE = 50                           # MLP tiles, 4 j's each (2 pairs side by side)


def _tile_js(m):
    """The 4 j's (a0, a1, b0, b1) covered by MLP tile m."""
    base = 8 * (m // 2) + 2 * (m % 2)
    return base, base + 1, base + 4, base + 5


# ---------------------------------------------------------------- blob layout
class _Cols:
    def __init__(self):
        self.c = 0
        self.slabs = {}

    def add(self, name, width):
        self.slabs[name] = (self.c, width)
        self.c += width

WB = _Cols()
WB.add("lhsT_x0", 256)     # 4 slots of Wih0.T at rows 32s..32s+6
WB.add("lhsT_h0", 256)     # rows 64:128 = Whh0.T
WB.add("lhsT_l1", 256)     # rows 0:64 = Whh1p.T, rows 64:128 = Wih1p.T
WB.add("identB", 128)      # bf16 identity
WB.add("lhsT_W1hi", 128)   # blockdiag(W1hi.T, W1hi.T)
WB.add("lhsT_W2", 64)      # blockdiag(W2.T, W2.T)
WB.add("lhsT_W3", 4)       # col q rows 32q:32q+32 = w3
WB.add("rhs_cbt", 64)      # rows 0:64 = W1hj.T, row 64 = mlp_b1
WB.add("rhs_ep0", 64)      # rows 0:5 = epW0.T, row 5 = epb0
WB.add("rhs_ep1", 64)
WB.add("rhs_gcnW0", 64)    # rows 0:64
WB.add("rhs_gcnW1", 64)
WB.add("rhs_gcnb0", 64)    # row 0
WB.add("rhs_gcnb1", 64)
WB.add("ones_row", 200)    # row 0 = 1.0
WB.add("lhsT_I2", 64)      # [I64; I64] (cross-half pair sum)
WB.add("bias1row", 256)    # row 0 = permuted layer-1 gate bias (g rows 2x)

WF = _Cols()
WF.add("biasL", 4)         # LSTM biases (g-rows pre-doubled): l0c0, l0c1, l1c0, l1c1
WF.add("scaleL", 2)        # ACT scale for chunk1: l0 = [2;1], l1 = [1;2]
WF.add("identF", 128)      # f32 identity
WF.add("b1col", 1)         # rows 0:64 = mlp_b1
WF.add("b2x4", 1)          # rows 0:128 = tile(mlp_b2, 4)
WF.add("b3col", 1)         # rows 0:128 = mlp_b3 (sigmoid bias)
WF.add("ln_g0", 64)        # row 0 (used via partition-broadcast DMA)
WF.add("ln_b0", 64)
WF.add("ln_g1", 64)
WF.add("ln_b1", 64)


def _pack_weights(inp):
    wb = np.zeros((128, WB.c), np.float32)
    wf = np.zeros((128, WF.c), np.float32)

    def put_b(name, rows, arr):
        c0, w = WB.slabs[name]
        wb[rows, c0:c0 + np.asarray(arr).shape[1]] = np.asarray(arr, np.float32)

    def put_f(name, rows, arr):
        c0, w = WF.slabs[name]
        wf[rows, c0:c0 + np.asarray(arr).shape[1]] = np.asarray(arr, np.float32)

    gsc = np.ones((256, 1), np.float32)
    gsc[128:192] = 2.0                       # g-gate rows computed pre-scaled
    Wih0, Whh0 = inp["Wih0"] * gsc, inp["Whh0"] * gsc
    b0 = inp["bih0"] + inp["bhh0"]
    # layer1 gate permutation: chunk0 = [f, i], chunk1 = [o, g]
    perm1 = np.concatenate([np.arange(64, 128), np.arange(0, 64),
                            np.arange(192, 256), np.arange(128, 192)])
    Wih1p = (inp["Wih1"] * gsc)[perm1]
    Whh1p = (inp["Whh1"] * gsc)[perm1]
    b1p = (inp["bih1"] + inp["bhh1"])[perm1]

    x0b = np.concatenate([Wih0.T, (b0 * gsc[:, 0])[None, :]], axis=0)   # [7, 256]
    for s in range(4):
        put_b("lhsT_x0", slice(32 * s, 32 * s + 7), x0b)
    put_b("bias1row", slice(0, 1), (b1p * np.concatenate(
        [np.ones(128), np.ones(64), np.full(64, 2.0)]))[None, :])
    put_b("lhsT_h0", slice(64, 128), Whh0.T)
    put_b("lhsT_l1", slice(0, 64), Whh1p.T)
    put_b("lhsT_l1", slice(64, 128), Wih1p.T)
    put_b("identB", slice(0, 128), np.eye(128))

    W1 = inp["mlp_W1"]                      # [64, 133]
    W1hi, W1hj = W1[:, :64], W1[:, 64:128]
    bd = np.zeros((128, 128), np.float32)
    bd[0:64, 0:64] = W1hi.T
    bd[64:128, 64:128] = W1hi.T
    put_b("lhsT_W1hi", slice(0, 128), bd)
    W2 = inp["mlp_W2"]                      # [32, 64]
    w2bd = np.zeros((128, 64), np.float32)
    w2bd[0:64, 0:32] = W2.T
    w2bd[64:128, 32:64] = W2.T
    put_b("lhsT_W2", slice(0, 128), w2bd)
    w3 = np.zeros((128, 4), np.float32)
    for q in range(4):
        w3[32 * q:32 * q + 32, q] = inp["mlp_W3"][0]
    put_b("lhsT_W3", slice(0, 128), w3)
    put_b("rhs_cbt", slice(0, 64), W1hj.T)
    put_b("rhs_cbt", slice(64, 65), inp["mlp_b1"][None, :])

    for l in range(2):
        ep = np.zeros((6, 64), np.float32)
        ep[0:5] = inp["ep_W"][l].T
        ep[5] = inp["ep_b"][l]
        put_b(f"rhs_ep{l}", slice(0, 6), ep)
        put_b(f"rhs_gcnW{l}", slice(0, 64), inp["gcn_W"][l].T)
        put_b(f"rhs_gcnb{l}", slice(0, 1), inp["gcn_b"][l][None, :])
    put_b("ones_row", slice(0, 1), np.ones((1, 200)))

    put_f("identF", slice(0, 128), np.eye(128))
    put_b("lhsT_I2", slice(0, 128), np.concatenate([np.eye(64), np.eye(64)], axis=0))
    put_f("b1col", slice(0, 64), inp["mlp_b1"][:, None])
    put_f("b2x4", slice(0, 128), np.tile(inp["mlp_b2"], 4)[:, None])
    put_f("b3col", slice(0, 128),
          np.full((128, 1), float(np.asarray(inp["mlp_b3"]).reshape(-1)[0]), np.float32))
    for l in range(2):
        put_f(f"ln_g{l}", slice(0, 1), inp["ln_g"][l][None, :])
        put_f(f"ln_b{l}", slice(0, 1), inp["ln_b"][l][None, :])

    return wb.astype(NPBF), wf


def _pack_embase(inp):
    """[12, NTILE*128] bf16: per-tile lhsT base for the K=12 mm1e.
    Rows 0:10 = blockdiag(W1e.T, W1e.T); rows 10:12 zero (Cb written on device)."""
    W1e = np.asarray(inp["mlp_W1"][:, 128:133], np.float32)  # [64, 5]
    blk = np.zeros((12, 128), np.float32)
    blk[0:5, 0:64] = W1e.T
    blk[5:10, 64:128] = W1e.T
    em = np.tile(blk, (1, NTILE))
    return em.astype(NPBF)


def _pack_core(inp, b):
    """Per-core (per-batch-element) data blobs."""
    edge = np.asarray(inp["edge_features"][b, -1], np.float32)   # [N, N, E]
    adj = np.asarray(inp["adjacency"][b], np.float32)            # [N, N]
    node = np.asarray(inp["node_features"][b], np.float32)       # [W, N, F]

    # xpad [128, 600]: tile k, slot s, rows 32s:32s+6 = node[4k+s].T
    xt = node.transpose(0, 2, 1)                                  # [W, F, N]
    xp = np.zeros((3, 4, 32, 200), np.float32)
    xp[:, :, 0:6] = xt.reshape(3, 4, 6, 200)
    xp[:, :, 6] = 1.0                        # ones row: layer-0 gate bias via K=7
    xpad_full = np.zeros((128, 600), np.float32)
    for s in range(4):
        for k in range(3):
            xpad_full[32 * s:32 * s + 32, 200 * k:200 * k + 200] = xp[k, s]

    # edge_w [128, 2000]: (i, e-major*j) layout, chunk1 in cols 1000:2000
    ew = edge.transpose(0, 2, 1).reshape(200, 1000)               # [i, e*200+j]
    edge_w = np.zeros((128, 2000), np.float32)
    edge_w[:, 0:1000] = ew[0:128]
    edge_w[0:72, 1000:2000] = ew[128:200]

    # edge_mlp [12, NTILE*400]: tile m cols 400m:400m+400 = pairs (a, b);
    # rows 0:5 = e of half-0 j, rows 5:10 = e of half-1 j, rows 10/11 =
    # indicator for pair a / b (selects the Cb lhsT rows).
    em = edge.transpose(1, 2, 0)                                  # [j, e, i]
    edge_mlp = np.zeros((12, NTILE * 400), np.float32)
    for m in range(NTILE):
        a0, a1, b0_, b1_ = _tile_js(m)
        c = 400 * m
        edge_mlp[0:5, c:c + 200] = em[a0]
        edge_mlp[5:10, c:c + 200] = em[a1]
        edge_mlp[0:5, c + 200:c + 400] = em[b0_]
        edge_mlp[5:10, c + 200:c + 400] = em[b1_]
        edge_mlp[10, c:c + 200] = 1.0
        edge_mlp[11, c + 200:c + 400] = 1.0

    return {
        "xpad": xpad_full.astype(NPBF),
        "edge_w": edge_w.astype(NPBF),
        "edge_mlp": edge_mlp.astype(NPBF),
        "adj": adj.astype(NPBF),
    }


# ---------------------------------------------------------------- bass program
def _build(debug=False):
    nc = bacc.Bacc("TRN2", target_bir_lowering=False)
    d = {}
    d["xpad"] = nc.dram_tensor("xpad", [128, 600], BF16, kind="ExternalInput").ap()
    d["edge_w"] = nc.dram_tensor("edge_w", [128, 2000], BF16, kind="ExternalInput").ap()
    d["edge_mlp"] = nc.dram_tensor("edge_mlp", [12, NTILE * 400], BF16,
                                   kind="ExternalInput").ap()
    d["embase"] = nc.dram_tensor("embase", [12, NTILE * 128], BF16,
                                 kind="ExternalInput").ap()
    d["adj"] = nc.dram_tensor("adj", [200, 200], BF16, kind="ExternalInput").ap()
    d["wb16"] = nc.dram_tensor("wb16", [128, WB.c], BF16, kind="ExternalInput").ap()
    d["wf32"] = nc.dram_tensor("wf32", [128, WF.c], F32, kind="ExternalInput").ap()
    d_out = nc.dram_tensor("outT", [200, 200], F32, kind="ExternalOutput").ap()
    d["cbst"] = nc.dram_tensor("cbst_scratch", [200, 64], BF16, kind="Internal").ap()
    dbg = {}
    if debug:
        for nm, shp, dt in [("dbg_h", [64, 200], BF16), ("dbg_h1", [64, 200], BF16),
                            ("dbg_h2", [64, 200], BF16), ("dbg_w6", [200, 6], F32),
                            ("dbg_cb", [128, 64], BF16)]:
            dbg[nm] = nc.dram_tensor(nm, shp, dt, kind="ExternalOutput").ap()

    with tile.TileContext(nc) as tc:
        _body(nc, tc, d, d_out, dbg)
    nc.compile()
    return nc


def _body(nc, tc, d, d_out, dbg):
    import contextlib
    ctx = contextlib.ExitStack()
    with ctx:
        consts = ctx.enter_context(tc.tile_pool(name="consts", bufs=1))
        work = ctx.enter_context(tc.tile_pool(name="work", bufs=3))

        # ---------------- constants + inputs (LSTM-critical ones first)
        wb = consts.tile([128, WB.c], BF16)
        nc.sync.dma_start(out=wb, in_=d["wb16"])
        xpad = consts.tile([128, 600], BF16)
        nc.sync.dma_start(out=xpad, in_=d["xpad"])
        wf = consts.tile([128, WF.c], F32)
        nc.sync.dma_start(out=wf, in_=d["wf32"])

        def WBS(name, rows=slice(0, 128)):
            c0, w = WB.slabs[name]
            return wb[rows, c0:c0 + w]

        def WFS(name, rows=slice(0, 128)):
            c0, w = WF.slabs[name]
            return wf[rows, c0:c0 + w]

        edge_w = consts.tile([128, 2000], BF16)
        nc.gpsimd.dma_start(out=edge_w, in_=d["edge_w"])
        edge_mlp = consts.tile([12, NTILE * 400], BF16)
        nc.gpsimd.dma_start(out=edge_mlp, in_=d["edge_mlp"])
        lhsT_em = consts.tile([12, NTILE * 128], BF16)
        nc.gpsimd.dma_start(out=lhsT_em, in_=d["embase"])

        # adj broadcast x5 (partition rows = i, free = (e, j) with e step 0)
        adjx = []
        for k, (i0, ck) in enumerate(CHUNKS):
            t = consts.tile([128, 5, 200], BF16, tag=f"adjx{k}")
            src = bass.AP(tensor=d["adj"].tensor, offset=i0 * 200,
                          ap=[[200, ck], [0, 5], [1, 200]])
            nc.sync.dma_start(out=t[0:ck], in_=src)
            adjx.append(t)

        eps_t = consts.tile([128, 1], F32)
        nc.vector.memset(eps_t, LN_EPS)

        # PE warmup + keepalive: the HAM clock-gate opens only after a ~3.4us
        # continuously-busy window and shuts after any ~3.4us idle window, so
        # burst at the start and sprinkle filler matmuls into every gap.
        ps_warm = ctx.enter_context(tc.tile_pool(name="ps_warm", bufs=1, space="PSUM"))
        pw = ps_warm.tile([128, 512], F32)

        def pe_fill(n, cols=256):
            for _ in range(n):
                nc.tensor.matmul(pw[:, 0:cols], wb[:, 0:128], wb[:, 0:cols],
                                 start=True, stop=True, skip_group_check=True)

        pe_fill(20, 512)

        # ---------------- wedge: wedge[i, e] = sum_j adj[i,j]*edge[i,j,e]
        w6 = []
        for k, (i0, ck) in enumerate(CHUNKS):
            tmp = work.tile([128, 1000], BF16, tag="wtmp")
            nc.gpsimd.tensor_mul(
                out=tmp[0:ck],
                in0=edge_w[0:ck, 1000 * k:1000 * k + 1000],
                in1=adjx[k][0:ck].rearrange("p e j -> p (e j)"),
            )
            w6c = consts.tile([128, 6], F32, tag=f"w6_{k}")
            for e in range(5):
                nc.vector.reduce_sum(out=w6c[0:ck, e:e + 1],
                                     in_=tmp[0:ck, 200 * e:200 * e + 200], axis=AX.X)
            nc.vector.reduce_sum(out=w6c[0:ck, 5:6],
                                 in_=adjx[k][0:ck, 0:1, :], axis=AX.X)
            w6.append(w6c)
        if dbg:
            nc.gpsimd.dma_start(out=dbg["dbg_w6"][0:128, :], in_=w6[0][0:128])
            nc.gpsimd.dma_start(out=dbg["dbg_w6"][128:200, :], in_=w6[1][0:72])

        # LN scale/bias broadcast tiles (only needed by the GCN)
        lng, lnb = [], []
        for l in range(2):
            g = consts.tile([128, 64], F32, tag=f"lng{l}")
            bt = consts.tile([128, 64], F32, tag=f"lnb{l}")
            for t, nm in ((g, f"ln_g{l}"), (bt, f"ln_b{l}")):
                c0, wdt = WF.slabs[nm]
                ap_b = bass.AP(tensor=d["wf32"].tensor, offset=c0, ap=[[0, 128], [1, 64]])
                nc.gpsimd.dma_start(out=t, in_=ap_b)
            lng.append(g)
            lnb.append(bt)

        # ---------------- LSTM (12 steps x 2 layers)
        # Gate layout per layer-step: one [128, 400] psum tile G;
        #   cols 0:200   = chunk0 (sigmoid gates)
        #   cols 200:400 = chunk1 (layer0: [g; o], layer1: [o; g])
        # chunk1 is evaluated with one sigmoid whose per-partition scale is 2
        # on the g rows; tanh(g) = 2*sigmoid(2g) - 1 is fixed up on gpsimd.
        hTfin = consts.tile([128, 200], BF16, tag="hTfin")
        with tc.tile_pool(name="ps_g", bufs=4, space="PSUM") as ps_g, \
             tc.tile_pool(name="ps_c", bufs=2, space="PSUM") as ps_c, \
             tc.tile_pool(name="lstm", bufs=4) as lp:
            Y = [lp.tile([128, 200], BF16, tag="Y", name=f"Y{t}") for t in range(12)] + [hTfin]
            c_prev = None
            for t in range(12):
                ct = ps_c.tile([128, 400], F32, tag="c")
                s = t % 4
                k = t // 4
                rhs_x = xpad[32 * s:32 * s + 7, 200 * k:200 * k + 200]
                # ---- layer 0: chunk0 = [i; f], chunk1 = [g; o]
                G0 = ps_g.tile([128, 400], F32, tag="g")
                for c in range(2):
                    sl = G0[:, 200 * c:200 * c + 200]
                    nc.tensor.matmul(sl, WBS("lhsT_x0", slice(32 * s, 32 * s + 7))[:, 128 * c:128 * c + 128],
                                     rhs_x, start=True, stop=(t == 0),
                                     tile_position=(32 * s, 0), skip_group_check=True)
                    if t > 0:
                        nc.tensor.matmul(sl, WBS("lhsT_h0", slice(64, 128))[:, 128 * c:128 * c + 128],
                                         Y[t - 1][64:128, :], start=False, stop=True,
                                         skip_group_check=True)
                # one sigmoid over all 4 gates (g rows arrive pre-scaled 2x);
                # S0 cols 0:200 = [si; sf], cols 200:400 = [sig(2g); so]
                S0 = lp.tile([128, 400], BF16, tag="S")
                nc.scalar.activation(out=S0, in_=G0, func=AF.Sigmoid)
                # tanh(g) = 2*sig(2g) - 1
                nc.vector.tensor_scalar(out=S0[0:64, 200:400], in0=S0[0:64, 200:400],
                                        scalar1=2.0, scalar2=-1.0, op0=ALU.mult, op1=ALU.add)
                M0 = lp.tile([128, 200], BF16, tag="M")
                nc.vector.tensor_mul(out=M0[0:64], in0=S0[0:64, 0:200], in1=S0[0:64, 200:400])
                if t > 0:
                    nc.vector.tensor_mul(out=M0[64:128], in0=S0[64:128, 0:200],
                                         in1=c_prev[64:128, 0:200])
                else:
                    nc.vector.memset(M0[64:128], 0.0)
                nc.tensor.matmul(ct[64:128, 0:200], WBS("lhsT_I2"), M0, start=True,
                                 stop=True, skip_group_check=True)
                pe_fill(4)
                TC0 = lp.tile([128, 200], BF16, tag="TC")
                nc.scalar.activation(out=TC0[64:128], in_=ct[64:128, 0:200], func=AF.Tanh)
                nc.vector.tensor_mul(out=Y[t][64:128], in0=S0[64:128, 200:400],
                                     in1=TC0[64:128])
                # ---- layer 1: chunk0 = [f; i], chunk1 = [o; g]
                G1 = ps_g.tile([128, 400], F32, tag="g")
                for c in range(2):
                    sl = G1[:, 200 * c:200 * c + 200]
                    if t == 0:
                        nc.tensor.matmul(sl, WBS("lhsT_l1", slice(64, 128))[:, 128 * c:128 * c + 128],
                                         Y[t][64:128, :], start=True, stop=False,
                                         skip_group_check=True)
                    else:
                        nc.tensor.matmul(sl, WBS("lhsT_l1")[:, 128 * c:128 * c + 128],
                                         Y[t], start=True, stop=False, skip_group_check=True)
                    nc.tensor.matmul(sl, WBS("bias1row", slice(0, 1))[:, 128 * c:128 * c + 128],
                                     WBS("ones_row", slice(0, 1)), start=False, stop=True,
                                     skip_group_check=True)
                # S1 cols 0:200 = [sf; si], cols 200:400 = [so; sig(2g)]
                S1 = lp.tile([128, 400], BF16, tag="S")
                nc.scalar.activation(out=S1, in_=G1, func=AF.Sigmoid)
                nc.vector.tensor_scalar(out=S1[64:128, 200:400], in0=S1[64:128, 200:400],
                                        scalar1=2.0, scalar2=-1.0, op0=ALU.mult, op1=ALU.add)
                M1 = lp.tile([128, 200], BF16, tag="M")
                if t > 0:
                    nc.vector.tensor_mul(out=M1[0:64], in0=S1[0:64, 0:200],
                                         in1=c_prev[0:64, 200:400])
                else:
                    nc.vector.memset(M1[0:64], 0.0)
                nc.vector.tensor_mul(out=M1[64:128], in0=S1[64:128, 0:200],
                                     in1=S1[64:128, 200:400])
                nc.tensor.matmul(ct[0:64, 200:400], WBS("lhsT_I2"), M1, start=True,
                                 stop=True, skip_group_check=True)
                pe_fill(4)
                TC1 = lp.tile([128, 200], BF16, tag="TC")
                nc.scalar.activation(out=TC1[0:64], in_=ct[0:64, 200:400], func=AF.Tanh)
                nc.vector.tensor_mul(out=Y[t + 1][0:64], in0=S1[0:64, 200:400],
                                     in1=TC1[0:64])
                c_prev = ct
        if dbg:
            nc.gpsimd.dma_start(out=dbg["dbg_h"], in_=hTfin[0:64])

        # ---------------- wedge transpose ([i,6] chunks -> wedgeT6 [6, 200])
        wedgeT6 = consts.tile([6, 200], BF16)
        with tc.tile_pool(name="ps_wt", bufs=2, space="PSUM") as ps_wt:
            for k, (i0, ck) in enumerate(CHUNKS):
                pwt = ps_wt.tile([6, 128], F32, tag="wt")
                nc.tensor.transpose(pwt[:, 0:ck], w6[k][0:ck], WFS("identF", slice(0, ck))[:, 0:ck])
                nc.vector.tensor_copy(out=wedgeT6[:, i0:i0 + ck], in_=pwt[:, 0:ck])

        # ---------------- GCN (2 layers)
        hT_cur = hTfin
        with tc.tile_pool(name="ps_u", bufs=2, space="PSUM") as ps_u, \
             tc.tile_pool(name="ps_t", bufs=2, space="PSUM") as ps_t:
            for l in range(2):
                hT_next = consts.tile([65, 200], BF16, tag=f"hT{l + 1}")
                nc.vector.memset(hT_next[64:65, :], 1.0)
                for k, (i0, ck) in enumerate(CHUNKS):
                    pu = ps_u.tile([128, 64], F32, tag="u")
                    nc.tensor.matmul(pu[0:ck], wedgeT6[:, i0:i0 + ck], WBS(f"rhs_ep{l}", slice(0, 6)),
                                     start=True, stop=False)
                    nc.tensor.matmul(pu[0:ck], hT_cur[0:64, i0:i0 + ck], WBS(f"rhs_gcnW{l}", slice(0, 64)),
                                     start=False, stop=False)
                    nc.tensor.matmul(pu[0:ck], WBS("ones_row", slice(0, 1))[:, i0:i0 + ck],
                                     WBS(f"rhs_gcnb{l}", slice(0, 1)), start=False, stop=True)
                    stats = work.tile([128, nc.vector.BN_STATS_DIM], F32, tag="bst")
                    nc.vector.bn_stats(out=stats[0:ck], in_=pu[0:ck])
                    mv = work.tile([128, nc.vector.BN_AGGR_DIM], F32, tag="mv")
                    nc.vector.bn_aggr(out=mv[0:ck], in_=stats[0:ck])
                    rstd = work.tile([128, 1], F32, tag="rstd")
                    nc.scalar.activation(out=rstd[0:ck], in_=mv[0:ck, 1:2], func=AF.Sqrt,
                                         bias=eps_t[0:ck])
                    nc.vector.reciprocal(out=rstd[0:ck], in_=rstd[0:ck])
                    xn = work.tile([128, 64], F32, tag="xn")
                    nc.vector.tensor_scalar(out=xn[0:ck], in0=pu[0:ck], scalar1=mv[0:ck, 0:1],
                                            scalar2=rstd[0:ck], op0=ALU.subtract, op1=ALU.mult)
                    nc.vector.tensor_mul(out=xn[0:ck], in0=xn[0:ck], in1=lng[l][0:ck])
                    nc.vector.tensor_add(out=xn[0:ck], in0=xn[0:ck], in1=lnb[l][0:ck])
                    hnew = work.tile([128, 64], BF16, tag="hnew")
                    nc.scalar.activation(out=hnew[0:ck], in_=xn[0:ck], func=AF.Relu)
                    pt = ps_t.tile([64, 128], BF16, tag="pt")
                    nc.tensor.transpose(pt[:, 0:ck], hnew[0:ck], WBS("identB", slice(0, ck))[:, 0:ck])
                    nc.vector.tensor_copy(out=hT_next[0:64, i0:i0 + ck], in_=pt[:, 0:ck])
                    pe_fill(8)
                hT_cur = hT_next
                if dbg:
                    nc.gpsimd.dma_start(out=dbg[f"dbg_h{l + 1}"], in_=hT_next[0:64])

        # ---------------- MLP prep: hT4, Cb -> CbS -> CbST -> lhsT_em rows 10:12
        hT4 = consts.tile([128, 400], BF16, tag="hT4")
        for r in range(2):
            for c in range(2):
                nc.sync.dma_start(out=hT4[64 * r:64 * r + 64, 200 * c:200 * c + 200],
                                  in_=hT_cur[0:64])
        # CbT[j, n] = (h2 @ W1hj.T + b1)[j, n], computed directly in j-major
        # layout: lhsT = [h2T; ones] (the GCN output tile already carries the
        # ones row), rhs = [W1hj.T; b1]. Then a 2-DMA DRAM gather drops rows
        # (a0, a0+1) and (b0, b0+1) into rows 10:12 of every lhsT_em block.
        CbTs = []
        with tc.tile_pool(name="ps_prep", bufs=2, space="PSUM") as ps_prep:
            for k, (i0, ck) in enumerate(CHUNKS):
                pcb = ps_prep.tile([128, 64], F32, tag="pcb")
                nc.tensor.matmul(pcb[0:ck], hT_cur[0:65, i0:i0 + ck],
                                 WBS("rhs_cbt", slice(0, 65)), start=True, stop=True)
                ct = consts.tile([128, 64], BF16, tag=f"cbts{k}")
                nc.vector.tensor_copy(out=ct[0:ck], in_=pcb[0:ck])
                CbTs.append(ct)
        nc.sync.dma_start(out=d["cbst"][0:128, :], in_=CbTs[0][0:128])
        nc.gpsimd.dma_start(out=d["cbst"][128:200, :], in_=CbTs[1][0:72])
        for x in range(2):
            # j(R, b) = 8b + 2x + 4R; cols 0:128 of each target row span CbT
            # rows j and j+1 contiguously in the DRAM scratch.
            gsrc = bass.AP(tensor=d["cbst"].tensor, offset=2 * x * 64,
                           ap=[[256, 2], [512, 25], [1, 128]])
            gdst_base = lhsT_em[10:12, :]
            gdst = bass.AP(tensor=gdst_base.tensor, offset=gdst_base.offset + 128 * x,
                           ap=[gdst_base.ap[0], [256, 25], [1, 128]])
            nc.sync.dma_start(out=gdst, in_=gsrc)
        if dbg:
            nc.gpsimd.dma_start(out=dbg["dbg_cb"], in_=CbTs[0])

        pe_fill(16, 512)   # re-open the clock gate before the MLP burst

        # ---------------- MLP main loop (50 tiles of 4 j's)
        with tc.tile_pool(name="ps1", bufs=4, space="PSUM") as ps1, \
             tc.tile_pool(name="ps2", bufs=2, space="PSUM") as ps2, \
             tc.tile_pool(name="ps3", bufs=1, space="PSUM") as ps3:
            p3 = ps3.tile([128, 400], F32)
            nc.vector.memset(p3, 0.0)
            p2 = None
            stage = None
            for m in range(NTILE):
                p1 = ps1.tile([128, 400], F32, tag="p1")
                nc.tensor.matmul(p1, WBS("lhsT_W1hi"), hT4, start=True, stop=False)
                nc.tensor.matmul(p1, lhsT_em[:, 128 * m:128 * m + 128],
                                 edge_mlp[:, 400 * m:400 * m + 400],
                                 start=False, stop=True)
                z1 = work.tile([128, 400], BF16, tag="z1")
                if m % 2 == 0:
                    nc.scalar.activation(out=z1, in_=p1, func=AF.Relu)
                else:
                    nc.vector.tensor_scalar_max(out=z1, in0=p1, scalar1=0.0)
                if m % 2 == 0:
                    p2 = ps2.tile([128, 400], F32, tag="p2")
                nc.tensor.matmul(p2[64 * (m % 2):64 * (m % 2) + 64, :],
                                 WBS("lhsT_W2"), z1, start=True, stop=True,
                                 skip_group_check=True)
                if m % 2 == 1:
                    v = m // 2
                    z2r = work.tile([128, 400], BF16, tag="z2")
                    nc.vector.tensor_scalar(out=z2r, in0=p2, scalar1=WFS("b2x4"),
                                            scalar2=0.0, op0=ALU.add, op1=ALU.max)
                    q = v % 4
                    nc.tensor.matmul(p3[32 * q:32 * q + 4, :], WBS("lhsT_W3"), z2r,
                                     start=True, stop=True, skip_group_check=True,
                                     tile_position=(0, 32 * q))
                    if q == 0:
                        stage = work.tile([128, 400], F32, tag="stage")
                    if q == 3 or v == 24:
                        nc.scalar.activation(out=stage, in_=p3, func=AF.Sigmoid,
                                             bias=WFS("b3col"))
                        for q2 in range(q + 1):
                            vv = (v // 4) * 4 + q2
                            dst = bass.AP(tensor=d_out.tensor, offset=8 * vv * 200,
                                          ap=[[200, 4], [800, 2], [1, 200]])
                            nc.sync.dma_start(
                                out=dst,
                                in_=stage[32 * q2:32 * q2 + 4, :].rearrange(
                                    "p (u i) -> p u i", u=2))


# ---------------------------------------------------------------- entry points
_CACHE = {}


def _get_nc(debug=False):
    key = bool(debug)
    if key not in _CACHE:
        _CACHE[key] = _build(debug)
    return _CACHE[key]


def _make_in_maps(inputs):
    wb, wf = _pack_weights(inputs)
    emb = _pack_embase(inputs)
    maps = []
    for b in range(B):
        m = _pack_core(inputs, b)
        m["wb16"] = wb
        m["wf32"] = wf
        m["embase"] = emb
        maps.append(m)
    return maps


def _run(inputs, trace=False, debug=False):
    nc = _get_nc(debug)
    in_maps = _make_in_maps(inputs)
    res = bass_utils.run_bass_kernel_spmd(nc, in_maps, core_ids=list(range(B)), trace=trace)
    outs = np.stack([res.results[b]["outT"].T for b in range(B)], axis=0).astype(np.float32)
    return outs, res


def kernel(**inputs):
    inputs = {k: np.asarray(v) for k, v in inputs.items()}
    outs, _ = _run(inputs, trace=False)
    return outs


# revision 23
# speedup vs baseline: 1.0598x; 1.0068x over previous
# Trainium2 Bass kernel for nn_TemporalGCN (LSTM -> 2x GCN -> pairwise edge MLP).
#
# Sharding: pure data-parallel over B (8 batch elements -> 8 NeuronCores).
# Each core runs an identical program on its own batch element; no collectives.
#
# Key algebraic simplification: the GCN message pass
#   messages = einsum('ij,ijn->in', adj, edge @ epW.T + epb)
# collapses to   wedge @ epW.T + rowsum(adj) x epb   with
#   wedge[i,e] = sum_j adj[i,j] * edge[i,j,e]   ([N, E] only).
# Only edge_features[:, -1] is ever used, so the big [B,W,N,N,E] tensor is
# sliced on the host and never shipped.
#
# MLP structure: each 400-column tile covers 2 j-pairs (4 j's); the per-pair
# bias C_j + b1 is folded into the K=12 edge matmul via two indicator rows,
# the W1hi @ h_i term is a second accumulating matmul, and the last two
# layers run block-diagonal (2 j's per 64 lhsT columns), so the final
# 1-feature layer emits 8 j-rows per matmul.

import numpy as np

import concourse.bass as bass
import concourse.bacc as bacc
import concourse.tile as tile
from concourse import mybir
from concourse import bass_utils

H, E, F, B, W, N = 64, 5, 6, 8, 12, 200
LN_EPS = 1e-5

F32 = mybir.dt.float32
BF16 = mybir.dt.bfloat16
NPBF = mybir.dt.np(BF16)
AF = mybir.ActivationFunctionType
ALU = mybir.AluOpType
AX = mybir.AxisListType

CHUNKS = [(0, 128), (128, 72)]      # destination-node chunks over i
NTILE = 50                           # MLP tiles, 4 j's each (2 pairs side by side)


def _tile_js(m):
    """The 4 j's (a0, a1, b0, b1) covered by MLP tile m."""
    base = 8 * (m // 2) + 2 * (m % 2)
    return base, base + 1, base + 4, base + 5


# ---------------------------------------------------------------- blob layout
class _Cols:
    def __init__(self):
        self.c = 0
        self.slabs = {}

    def add(self, name, width):
        self.slabs[name] = (self.c, width)
        self.c += width

WB = _Cols()
WB.add("lhsT_x0", 256)     # 4 slots of Wih0.T at rows 32s..32s+6
WB.add("lhsT_h0", 256)     # rows 64:128 = Whh0.T
WB.add("lhsT_l1", 256)     # rows 0:64 = Whh1p.T, rows 64:128 = Wih1p.T
WB.add("identB", 128)      # bf16 identity
WB.add("lhsT_W1hi", 128)   # blockdiag(W1hi.T, W1hi.T)
WB.add("lhsT_W2", 64)      # blockdiag(W2.T, W2.T)
WB.add("lhsT_W3", 4)       # col q rows 32q:32q+32 = w3
WB.add("rhs_cbt", 64)      # rows 0:64 = W1hj.T, row 64 = mlp_b1
WB.add("rhs_ep0", 64)      # rows 0:5 = epW0.T, row 5 = epb0
WB.add("rhs_ep1", 64)
WB.add("rhs_gcnW0", 64)    # rows 0:64
WB.add("rhs_gcnW1", 64)
WB.add("rhs_gcnb0", 64)    # row 0
WB.add("rhs_gcnb1", 64)
WB.add("ones_row", 200)    # row 0 = 1.0
WB.add("lhsT_I2", 64)      # [I64; I64] (cross-half pair sum)
WB.add("bias1row", 256)    # row 0 = permuted layer-1 gate bias (g rows 2x)

WF = _Cols()
WF.add("biasL", 4)         # LSTM biases (g-rows pre-doubled): l0c0, l0c1, l1c0, l1c1
WF.add("scaleL", 2)        # ACT scale for chunk1: l0 = [2;1], l1 = [1;2]
WF.add("identF", 128)      # f32 identity
WF.add("b1col", 1)         # rows 0:64 = mlp_b1
WF.add("b2x4", 1)          # rows 0:128 = tile(mlp_b2, 4)
WF.add("b3col", 1)         # rows 0:128 = mlp_b3 (sigmoid bias)
WF.add("ln_g0", 64)        # row 0 (used via partition-broadcast DMA)
WF.add("ln_b0", 64)
WF.add("ln_g1", 64)
WF.add("ln_b1", 64)


def _pack_weights(inp):
    wb = np.zeros((128, WB.c), np.float32)
    wf = np.zeros((128, WF.c), np.float32)

    def put_b(name, rows, arr):
        c0, w = WB.slabs[name]
        wb[rows, c0:c0 + np.asarray(arr).shape[1]] = np.asarray(arr, np.float32)

    def put_f(name, rows, arr):
        c0, w = WF.slabs[name]
        wf[rows, c0:c0 + np.asarray(arr).shape[1]] = np.asarray(arr, np.float32)

    gsc = np.ones((256, 1), np.float32)
    gsc[128:192] = 2.0                       # g-gate rows computed pre-scaled
    Wih0, Whh0 = inp["Wih0"] * gsc, inp["Whh0"] * gsc
    b0 = inp["bih0"] + inp["bhh0"]
    # layer1 gate permutation: chunk0 = [f, i], chunk1 = [o, g]
    perm1 = np.concatenate([np.arange(64, 128), np.arange(0, 64),
                            np.arange(192, 256), np.arange(128, 192)])
    Wih1p = (inp["Wih1"] * gsc)[perm1]
    Whh1p = (inp["Whh1"] * gsc)[perm1]
    b1p = (inp["bih1"] + inp["bhh1"])[perm1]

    x0b = np.concatenate([Wih0.T, (b0 * gsc[:, 0])[None, :]], axis=0)   # [7, 256]
    for s in range(4):
        put_b("lhsT_x0", slice(32 * s, 32 * s + 7), x0b)
    put_b("bias1row", slice(0, 1), (b1p * np.concatenate(
        [np.ones(128), np.ones(64), np.full(64, 2.0)]))[None, :])
    put_b("lhsT_h0", slice(64, 128), Whh0.T)
    put_b("lhsT_l1", slice(0, 64), Whh1p.T)
    put_b("lhsT_l1", slice(64, 128), Wih1p.T)
    put_b("identB", slice(0, 128), np.eye(128))

    W1 = inp["mlp_W1"]                      # [64, 133]
    W1hi, W1hj = W1[:, :64], W1[:, 64:128]
    bd = np.zeros((128, 128), np.float32)
    bd[0:64, 0:64] = W1hi.T
    bd[64:128, 64:128] = W1hi.T
    put_b("lhsT_W1hi", slice(0, 128), bd)
    W2 = inp["mlp_W2"]                      # [32, 64]
    w2bd = np.zeros((128, 64), np.float32)
    w2bd[0:64, 0:32] = W2.T
    w2bd[64:128, 32:64] = W2.T
    put_b("lhsT_W2", slice(0, 128), w2bd)
    w3 = np.zeros((128, 4), np.float32)
    for q in range(4):
        w3[32 * q:32 * q + 32, q] = inp["mlp_W3"][0]
    put_b("lhsT_W3", slice(0, 128), w3)
    put_b("rhs_cbt", slice(0, 64), W1hj.T)
    put_b("rhs_cbt", slice(64, 65), inp["mlp_b1"][None, :])

    for l in range(2):
        ep = np.zeros((6, 64), np.float32)
        ep[0:5] = inp["ep_W"][l].T
        ep[5] = inp["ep_b"][l]
        put_b(f"rhs_ep{l}", slice(0, 6), ep)
        put_b(f"rhs_gcnW{l}", slice(0, 64), inp["gcn_W"][l].T)
        put_b(f"rhs_gcnb{l}", slice(0, 1), inp["gcn_b"][l][None, :])
    put_b("ones_row", slice(0, 1), np.ones((1, 200)))

    put_f("identF", slice(0, 128), np.eye(128))
    put_b("lhsT_I2", slice(0, 128), np.concatenate([np.eye(64), np.eye(64)], axis=0))
    put_f("b1col", slice(0, 64), inp["mlp_b1"][:, None])
    put_f("b2x4", slice(0, 128), np.tile(inp["mlp_b2"], 4)[:, None])
    put_f("b3col", slice(0, 128),
          np.full((128, 1), float(np.asarray(inp["mlp_b3"]).reshape(-1)[0]), np.float32))
    for l in range(2):
        put_f(f"ln_g{l}", slice(0, 1), inp["ln_g"][l][None, :])
        put_f(f"ln_b{l}", slice(0, 1), inp["ln_b"][l][None, :])

    return wb.astype(NPBF), wf


def _pack_embase(inp):
    """[12, NTILE*128] bf16: per-tile lhsT base for the K=12 mm1e.
    Rows 0:10 = blockdiag(W1e.T, W1e.T); rows 10:12 zero (Cb written on device)."""
    W1e = np.asarray(inp["mlp_W1"][:, 128:133], np.float32)  # [64, 5]
    blk = np.zeros((12, 128), np.float32)
    blk[0:5, 0:64] = W1e.T
    blk[5:10, 64:128] = W1e.T
    em = np.tile(blk, (1, NTILE))
    return em.astype(NPBF)


def _pack_core(inp, b):
    """Per-core (per-batch-element) data blobs."""
    edge = np.asarray(inp["edge_features"][b, -1], np.float32)   # [N, N, E]
    adj = np.asarray(inp["adjacency"][b], np.float32)            # [N, N]
    node = np.asarray(inp["node_features"][b], np.float32)       # [W, N, F]

    # xpad [128, 600]: tile k, slot s, rows 32s:32s+6 = node[4k+s].T
    xt = node.transpose(0, 2, 1)                                  # [W, F, N]
    xp = np.zeros((3, 4, 32, 200), np.float32)
    xp[:, :, 0:6] = xt.reshape(3, 4, 6, 200)
    xp[:, :, 6] = 1.0                        # ones row: layer-0 gate bias via K=7
    xpad_full = np.zeros((128, 600), np.float32)
    for s in range(4):
        for k in range(3):
            xpad_full[32 * s:32 * s + 32, 200 * k:200 * k + 200] = xp[k, s]

    # edge_w [128, 2000]: (i, e-major*j) layout, chunk1 in cols 1000:2000
    ew = edge.transpose(0, 2, 1).reshape(200, 1000)               # [i, e*200+j]
    edge_w = np.zeros((128, 2000), np.float32)
    edge_w[:, 0:1000] = ew[0:128]
    edge_w[0:72, 1000:2000] = ew[128:200]

    # edge_mlp [12, NTILE*400]: tile m cols 400m:400m+400 = pairs (a, b);
    # rows 0:5 = e of half-0 j, rows 5:10 = e of half-1 j, rows 10/11 =
    # indicator for pair a / b (selects the Cb lhsT rows).
    em = edge.transpose(1, 2, 0)                                  # [j, e, i]
    edge_mlp = np.zeros((12, NTILE * 400), np.float32)
    for m in range(NTILE):
        a0, a1, b0_, b1_ = _tile_js(m)
        c = 400 * m
        edge_mlp[0:5, c:c + 200] = em[a0]
        edge_mlp[5:10, c:c + 200] = em[a1]
        edge_mlp[0:5, c + 200:c + 400] = em[b0_]
        edge_mlp[5:10, c + 200:c + 400] = em[b1_]
        edge_mlp[10, c:c + 200] = 1.0
        edge_mlp[11, c + 200:c + 400] = 1.0

    return {
        "xpad": xpad_full.astype(NPBF),
        "edge_w": edge_w.astype(NPBF),
        "edge_mlp": edge_mlp.astype(NPBF),
        "adj": adj.astype(NPBF),
    }


# ---------------------------------------------------------------- bass program
def _build(debug=False):
    nc = bacc.Bacc("TRN2", target_bir_lowering=False)
    d = {}
    d["xpad"] = nc.dram_tensor("xpad", [128, 600], BF16, kind="ExternalInput").ap()
    d["edge_w"] = nc.dram_tensor("edge_w", [128, 2000], BF16, kind="ExternalInput").ap()
    d["edge_mlp"] = nc.dram_tensor("edge_mlp", [12, NTILE * 400], BF16,
                                   kind="ExternalInput").ap()
    d["embase"] = nc.dram_tensor("embase", [12, NTILE * 128], BF16,
                                 kind="ExternalInput").ap()
    d["adj"] = nc.dram_tensor("adj", [200, 200], BF16, kind="ExternalInput").ap()
    d["wb16"] = nc.dram_tensor("wb16", [128, WB.c], BF16, kind="ExternalInput").ap()
    d["wf32"] = nc.dram_tensor("wf32", [128, WF.c], F32, kind="ExternalInput").ap()
    d_out = nc.dram_tensor("outT", [200, 200], F32, kind="ExternalOutput").ap()
    d["cbst"] = nc.dram_tensor("cbst_scratch", [200, 64], BF16, kind="Internal").ap()
    dbg = {}
    if debug:
        for nm, shp, dt in [("dbg_h", [64, 200], BF16), ("dbg_h1", [64, 200], BF16),
                            ("dbg_h2", [64, 200], BF16), ("dbg_w6", [200, 6], F32),
                            ("dbg_cb", [128, 64], BF16)]:
            dbg[nm] = nc.dram_tensor(nm, shp, dt, kind="ExternalOutput").ap()

    with tile.TileContext(nc) as tc:
        _body(nc, tc, d, d_out, dbg)
    nc.compile()
    return nc


def _body(nc, tc, d, d_out, dbg):
    import contextlib
    ctx = contextlib.ExitStack()
    with ctx:
        consts = ctx.enter_context(tc.tile_pool(name="consts", bufs=1))
        work = ctx.enter_context(tc.tile_pool(name="work", bufs=3))

        # ---------------- constants + inputs (LSTM-critical ones first)
        wb = consts.tile([128, WB.c], BF16)
        nc.sync.dma_start(out=wb, in_=d["wb16"])
        xpad = consts.tile([128, 600], BF16)
        nc.sync.dma_start(out=xpad, in_=d["xpad"])
        wf = consts.tile([128, WF.c], F32)
        nc.sync.dma_start(out=wf, in_=d["wf32"])

        def WBS(name, rows=slice(0, 128)):
            c0, w = WB.slabs[name]
            return wb[rows, c0:c0 + w]

        def WFS(name, rows=slice(0, 128)):
            c0, w = WF.slabs[name]
            return wf[rows, c0:c0 + w]

        edge_w = consts.tile([128, 2000], BF16)
        nc.gpsimd.dma_start(out=edge_w, in_=d["edge_w"])
        edge_mlp = consts.tile([12, NTILE * 400], BF16)
        nc.gpsimd.dma_start(out=edge_mlp, in_=d["edge_mlp"])
        lhsT_em = consts.tile([12, NTILE * 128], BF16)
        nc.gpsimd.dma_start(out=lhsT_em, in_=d["embase"])

        # adj broadcast x5 (partition rows = i, free = (e, j) with e step 0)
        adjx = []
        for k, (i0, ck) in enumerate(CHUNKS):
            t = consts.tile([128, 5, 200], BF16, tag=f"adjx{k}")
            src = bass.AP(tensor=d["adj"].tensor, offset=i0 * 200,
                          ap=[[200, ck], [0, 5], [1, 200]])
            nc.sync.dma_start(out=t[0:ck], in_=src)
            adjx.append(t)

        eps_t = consts.tile([128, 1], F32)
        nc.vector.memset(eps_t, LN_EPS)

        # PE warmup + keepalive: the HAM clock-gate opens only after a ~3.4us
        # continuously-busy window and shuts after any ~3.4us idle window, so
        # burst at the start and sprinkle filler matmuls into every gap.
        ps_warm = ctx.enter_context(tc.tile_pool(name="ps_warm", bufs=1, space="PSUM"))
        pw = ps_warm.tile([128, 512], F32)

        def pe_fill(n, cols=256):
            for _ in range(n):
                nc.tensor.matmul(pw[:, 0:cols], wb[:, 0:128], wb[:, 0:cols],
                                 start=True, stop=True, skip_group_check=True)

        pe_fill(20, 512)

        # ---------------- wedge: wedge[i, e] = sum_j adj[i,j]*edge[i,j,e]
        w6 = []
        for k, (i0, ck) in enumerate(CHUNKS):
            tmp = work.tile([128, 1000], BF16, tag="wtmp")
            nc.gpsimd.tensor_mul(
                out=tmp[0:ck],
                in0=edge_w[0:ck, 1000 * k:1000 * k + 1000],
                in1=adjx[k][0:ck].rearrange("p e j -> p (e j)"),
            )
            w6c = consts.tile([128, 6], F32, tag=f"w6_{k}")
            for e in range(5):
                nc.vector.reduce_sum(out=w6c[0:ck, e:e + 1],
                                     in_=tmp[0:ck, 200 * e:200 * e + 200], axis=AX.X)
            nc.vector.reduce_sum(out=w6c[0:ck, 5:6],
                                 in_=adjx[k][0:ck, 0:1, :], axis=AX.X)
            w6.append(w6c)
        if dbg:
            nc.gpsimd.dma_start(out=dbg["dbg_w6"][0:128, :], in_=w6[0][0:128])
            nc.gpsimd.dma_start(out=dbg["dbg_w6"][128:200, :], in_=w6[1][0:72])

        # LN scale/bias broadcast tiles (only needed by the GCN)
        lng, lnb = [], []
        for l in range(2):
            g = consts.tile([128, 64], F32, tag=f"lng{l}")
            bt = consts.tile([128, 64], F32, tag=f"lnb{l}")
            for t, nm in ((g, f"ln_g{l}"), (bt, f"ln_b{l}")):
                c0, wdt = WF.slabs[nm]
                ap_b = bass.AP(tensor=d["wf32"].tensor, offset=c0, ap=[[0, 128], [1, 64]])
                nc.gpsimd.dma_start(out=t, in_=ap_b)
            lng.append(g)
            lnb.append(bt)

        # ---------------- LSTM (12 steps x 2 layers)
        # Gate layout per layer-step: one [128, 400] psum tile G;
        #   cols 0:200   = chunk0 (sigmoid gates)
        #   cols 200:400 = chunk1 (layer0: [g; o], layer1: [o; g])
        # chunk1 is evaluated with one sigmoid whose per-partition scale is 2
        # on the g rows; tanh(g) = 2*sigmoid(2g) - 1 is fixed up on gpsimd.
        hTfin = consts.tile([128, 200], BF16, tag="hTfin")
        with tc.tile_pool(name="ps_g", bufs=4, space="PSUM") as ps_g, \
             tc.tile_pool(name="ps_c", bufs=2, space="PSUM") as ps_c, \
             tc.tile_pool(name="lstm", bufs=4) as lp:
            Y = [lp.tile([128, 200], BF16, tag="Y", name=f"Y{t}") for t in range(12)] + [hTfin]
            c_prev = None
            for t in range(12):
                ct = ps_c.tile([128, 400], F32, tag="c")
                s = t % 4
                k = t // 4
                rhs_x = xpad[32 * s:32 * s + 7, 200 * k:200 * k + 200]
                # ---- layer 0: chunk0 = [i; f], chunk1 = [g; o]
                G0 = ps_g.tile([128, 400], F32, tag="g")
                for c in range(2):
                    sl = G0[:, 200 * c:200 * c + 200]
                    nc.tensor.matmul(sl, WBS("lhsT_x0", slice(32 * s, 32 * s + 7))[:, 128 * c:128 * c + 128],
                                     rhs_x, start=True, stop=(t == 0),
                                     tile_position=(32 * s, 0), skip_group_check=True)
                    if t > 0:
                        nc.tensor.matmul(sl, WBS("lhsT_h0", slice(64, 128))[:, 128 * c:128 * c + 128],
                                         Y[t - 1][64:128, :], start=False, stop=True,
                                         skip_group_check=True)
                # one sigmoid over all 4 gates (g rows arrive pre-scaled 2x);
                # S0 cols 0:200 = [si; sf], cols 200:400 = [sig(2g); so]
                S0 = lp.tile([128, 400], BF16, tag="S")
                nc.scalar.activation(out=S0, in_=G0, func=AF.Sigmoid)
                # tanh(g) = 2*sig(2g) - 1
                nc.vector.tensor_scalar(out=S0[0:64, 200:400], in0=S0[0:64, 200:400],
                                        scalar1=2.0, scalar2=-1.0, op0=ALU.mult, op1=ALU.add)
                M0 = lp.tile([128, 200], BF16, tag="M")
                nc.vector.tensor_mul(out=M0[0:64], in0=S0[0:64, 0:200], in1=S0[0:64, 200:400])
                if t > 0:
                    nc.vector.tensor_mul(out=M0[64:128], in0=S0[64:128, 0:200],
                                         in1=c_prev[64:128, 0:200])
                else:
                    nc.vector.memset(M0[64:128], 0.0)
                nc.tensor.matmul(ct[64:128, 0:200], WBS("lhsT_I2"), M0, start=True,
                                 stop=True, skip_group_check=True)
                pe_fill(4)
                TC0 = lp.tile([128, 200], BF16, tag="TC")
                nc.scalar.activation(out=TC0[64:128], in_=ct[64:128, 0:200], func=AF.Tanh)
                nc.vector.tensor_mul(out=Y[t][64:128], in0=S0[64:128, 200:400],
                                     in1=TC0[64:128])
                # ---- layer 1: chunk0 = [f; i], chunk1 = [o; g]
                G1 = ps_g.tile([128, 400], F32, tag="g")
                for c in range(2):
                    sl = G1[:, 200 * c:200 * c + 200]
                    if t == 0:
                        nc.tensor.matmul(sl, WBS("lhsT_l1", slice(64, 128))[:, 128 * c:128 * c + 128],
                                         Y[t][64:128, :], start=True, stop=False,
                                         skip_group_check=True)
                    else:
                        nc.tensor.matmul(sl, WBS("lhsT_l1")[:, 128 * c:128 * c + 128],
                                         Y[t], start=True, stop=False, skip_group_check=True)
                    nc.tensor.matmul(sl, WBS("bias1row", slice(0, 1))[:, 128 * c:128 * c + 128],
                                     WBS("ones_row", slice(0, 1)), start=False, stop=True,
                                     skip_group_check=True)
                # S1 cols 0:200 = [sf; si], cols 200:400 = [so; sig(2g)]
                S1 = lp.tile([128, 400], BF16, tag="S")
                nc.scalar.activation(out=S1, in_=G1, func=AF.Sigmoid)
                nc.vector.tensor_scalar(out=S1[64:128, 200:400], in0=S1[64:128, 200:400],
                                        scalar1=2.0, scalar2=-1.0, op0=ALU.mult, op1=ALU.add)
                M1 = lp.tile([128, 200], BF16, tag="M")
                if t > 0:
                    nc.vector.tensor_mul(out=M1[0:64], in0=S1[0:64, 0:200],
                                         in1=c_prev[0:64, 200:400])
                else:
                    nc.vector.memset(M1[0:64], 0.0)
                nc.vector.tensor_mul(out=M1[64:128], in0=S1[64:128, 0:200],
                                     in1=S1[64:128, 200:400])
                nc.tensor.matmul(ct[0:64, 200:400], WBS("lhsT_I2"), M1, start=True,
                                 stop=True, skip_group_check=True)
                pe_fill(4)
                TC1 = lp.tile([128, 200], BF16, tag="TC")
                nc.scalar.activation(out=TC1[0:64], in_=ct[0:64, 200:400], func=AF.Tanh)
                nc.vector.tensor_mul(out=Y[t + 1][0:64], in0=S1[0:64, 200:400],
                                     in1=TC1[0:64])
                c_prev = ct
        if dbg:
            nc.gpsimd.dma_start(out=dbg["dbg_h"], in_=hTfin[0:64])

        # ---------------- wedge transpose ([i,6] chunks -> wedgeT6 [6, 200])
        wedgeT6 = consts.tile([6, 200], BF16)
        with tc.tile_pool(name="ps_wt", bufs=2, space="PSUM") as ps_wt:
            for k, (i0, ck) in enumerate(CHUNKS):
                pwt = ps_wt.tile([6, 128], F32, tag="wt")
                nc.tensor.transpose(pwt[:, 0:ck], w6[k][0:ck], WFS("identF", slice(0, ck))[:, 0:ck])
                nc.vector.tensor_copy(out=wedgeT6[:, i0:i0 + ck], in_=pwt[:, 0:ck])

        # ---------------- GCN (2 layers)
        hT_cur = hTfin
        with tc.tile_pool(name="ps_u", bufs=2, space="PSUM") as ps_u, \
             tc.tile_pool(name="ps_t", bufs=2, space="PSUM") as ps_t:
            for l in range(2):
                hT_next = consts.tile([65, 200], BF16, tag=f"hT{l + 1}")
                nc.vector.memset(hT_next[64:65, :], 1.0)
                for k, (i0, ck) in enumerate(CHUNKS):
                    pu = ps_u.tile([128, 64], F32, tag="u")
                    nc.tensor.matmul(pu[0:ck], wedgeT6[:, i0:i0 + ck], WBS(f"rhs_ep{l}", slice(0, 6)),
                                     start=True, stop=False)
                    nc.tensor.matmul(pu[0:ck], hT_cur[0:64, i0:i0 + ck], WBS(f"rhs_gcnW{l}", slice(0, 64)),
                                     start=False, stop=False)
                    nc.tensor.matmul(pu[0:ck], WBS("ones_row", slice(0, 1))[:, i0:i0 + ck],
                                     WBS(f"rhs_gcnb{l}", slice(0, 1)), start=False, stop=True)
                    stats = work.tile([128, nc.vector.BN_STATS_DIM], F32, tag="bst")
                    nc.vector.bn_stats(out=stats[0:ck], in_=pu[0:ck])
                    mv = work.tile([128, nc.vector.BN_AGGR_DIM], F32, tag="mv")
                    nc.vector.bn_aggr(out=mv[0:ck], in_=stats[0:ck])
                    rstd = work.tile([128, 1], F32, tag="rstd")
                    nc.scalar.activation(out=rstd[0:ck], in_=mv[0:ck, 1:2], func=AF.Sqrt,
                                         bias=eps_t[0:ck])
                    nc.vector.reciprocal(out=rstd[0:ck], in_=rstd[0:ck])
                    xn = work.tile([128, 64], F32, tag="xn")
                    nc.vector.tensor_scalar(out=xn[0:ck], in0=pu[0:ck], scalar1=mv[0:ck, 0:1],
                                            scalar2=rstd[0:ck], op0=ALU.subtract, op1=ALU.mult)
                    nc.vector.tensor_mul(out=xn[0:ck], in0=xn[0:ck], in1=lng[l][0:ck])
                    nc.vector.tensor_add(out=xn[0:ck], in0=xn[0:ck], in1=lnb[l][0:ck])
                    hnew = work.tile([128, 64], BF16, tag="hnew")
                    nc.scalar.activation(out=hnew[0:ck], in_=xn[0:ck], func=AF.Relu)
                    pt = ps_t.tile([64, 128], BF16, tag="pt")
                    nc.tensor.transpose(pt[:, 0:ck], hnew[0:ck], WBS("identB", slice(0, ck))[:, 0:ck])
                    nc.vector.tensor_copy(out=hT_next[0:64, i0:i0 + ck], in_=pt[:, 0:ck])
                    pe_fill(8)
                hT_cur = hT_next
                if dbg:
                    nc.gpsimd.dma_start(out=dbg[f"dbg_h{l + 1}"], in_=hT_next[0:64])

        # ---------------- MLP prep: hT4, Cb -> CbS -> CbST -> lhsT_em rows 10:12
        hT4 = consts.tile([128, 400], BF16, tag="hT4")
        for r in range(2):
            for c in range(2):
                nc.sync.dma_start(out=hT4[64 * r:64 * r + 64, 200 * c:200 * c + 200],
                                  in_=hT_cur[0:64])
        # CbT[j, n] = (h2 @ W1hj.T + b1)[j, n], computed directly in j-major
        # layout: lhsT = [h2T; ones] (the GCN output tile already carries the
        # ones row), rhs = [W1hj.T; b1]. Then a 2-DMA DRAM gather drops rows
        # (a0, a0+1) and (b0, b0+1) into rows 10:12 of every lhsT_em block.
        CbTs = []
        with tc.tile_pool(name="ps_prep", bufs=2, space="PSUM") as ps_prep:
            for k, (i0, ck) in enumerate(CHUNKS):
                pcb = ps_prep.tile([128, 64], F32, tag="pcb")
                nc.tensor.matmul(pcb[0:ck], hT_cur[0:65, i0:i0 + ck],
                                 WBS("rhs_cbt", slice(0, 65)), start=True, stop=True)
                ct = consts.tile([128, 64], BF16, tag=f"cbts{k}")
                nc.vector.tensor_copy(out=ct[0:ck], in_=pcb[0:ck])
                CbTs.append(ct)
        nc.sync.dma_start(out=d["cbst"][0:128, :], in_=CbTs[0][0:128])
        nc.gpsimd.dma_start(out=d["cbst"][128:200, :], in_=CbTs[1][0:72])
        for x in range(2):
            # j(R, b) = 8b + 2x + 4R; cols 0:128 of each target row span CbT
            # rows j and j+1 contiguously in the DRAM scratch.
            gsrc = bass.AP(tensor=d["cbst"].tensor, offset=2 * x * 64,
                           ap=[[256, 2], [512, 25], [1, 128]])
            gdst_base = lhsT_em[10:12, :]
            gdst = bass.AP(tensor=gdst_base.tensor, offset=gdst_base.offset + 128 * x,
                           ap=[gdst_base.ap[0], [256, 25], [1, 128]])
            nc.sync.dma_start(out=gdst, in_=gsrc)
        if dbg:
            nc.gpsimd.dma_start(out=dbg["dbg_cb"], in_=CbTs[0])

        pe_fill(16, 512)   # re-open the clock gate before the MLP burst

        # ---------------- MLP main loop (50 tiles of 4 j's)
        with tc.tile_pool(name="ps1", bufs=4, space="PSUM") as ps1, \
             tc.tile_pool(name="ps2", bufs=2, space="PSUM") as ps2, \
             tc.tile_pool(name="ps3", bufs=1, space="PSUM") as ps3:
            p3 = ps3.tile([128, 400], F32)
            nc.vector.memset(p3, 0.0)
            p2 = None
            stage = None
            for m in range(NTILE):
                p1 = ps1.tile([128, 400], F32, tag="p1")
                nc.tensor.matmul(p1, WBS("lhsT_W1hi"), hT4, start=True, stop=False)
                if m == 0:
                    # dense burst pinned between the first two real matmuls:
                    # PE is in-order, so this runs contiguously right here,
                    # re-opening the clock gate while the Cb gather DMAs land.
                    pe_fill(12, 512)
                nc.tensor.matmul(p1, lhsT_em[:, 128 * m:128 * m + 128],
                                 edge_mlp[:, 400 * m:400 * m + 400],
                                 start=False, stop=True)
                z1 = work.tile([128, 400], BF16, tag="z1")
                if m % 2 == 0:
                    nc.scalar.activation(out=z1, in_=p1, func=AF.Relu)
                else:
                    nc.vector.tensor_scalar_max(out=z1, in0=p1, scalar1=0.0)
                if m % 2 == 0:
                    p2 = ps2.tile([128, 400], F32, tag="p2")
                nc.tensor.matmul(p2[64 * (m % 2):64 * (m % 2) + 64, :],
                                 WBS("lhsT_W2"), z1, start=True, stop=True,
                                 skip_group_check=True)
                if m % 2 == 1:
                    v = m // 2
                    z2r = work.tile([128, 400], BF16, tag="z2")
                    nc.vector.tensor_scalar(out=z2r, in0=p2, scalar1=WFS("b2x4"),
                                            scalar2=0.0, op0=ALU.add, op1=ALU.max)
                    q = v % 4
                    nc.tensor.matmul(p3[32 * q:32 * q + 4, :], WBS("lhsT_W3"), z2r,
                                     start=True, stop=True, skip_group_check=True,
                                     tile_position=(0, 32 * q))
                    if q == 0:
                        stage = work.tile([128, 400], F32, tag="stage")
                    if q == 3 or v == 24:
                        nc.scalar.activation(out=stage, in_=p3, func=AF.Sigmoid,
                                             bias=WFS("b3col"))
                        for q2 in range(q + 1):
                            vv = (v // 4) * 4 + q2
                            dst = bass.AP(tensor=d_out.tensor, offset=8 * vv * 200,
                                          ap=[[200, 4], [800, 2], [1, 200]])
                            nc.sync.dma_start(
                                out=dst,
                                in_=stage[32 * q2:32 * q2 + 4, :].rearrange(
                                    "p (u i) -> p u i", u=2))


# ---------------------------------------------------------------- entry points
_CACHE = {}


def _get_nc(debug=False):
    key = bool(debug)
    if key not in _CACHE:
        _CACHE[key] = _build(debug)
    return _CACHE[key]


def _make_in_maps(inputs):
    wb, wf = _pack_weights(inputs)
    emb = _pack_embase(inputs)
    maps = []
    for b in range(B):
        m = _pack_core(inputs, b)
        m["wb16"] = wb
        m["wf32"] = wf
        m["embase"] = emb
        maps.append(m)
    return maps


def _run(inputs, trace=False, debug=False):
    nc = _get_nc(debug)
    in_maps = _make_in_maps(inputs)
    res = bass_utils.run_bass_kernel_spmd(nc, in_maps, core_ids=list(range(B)), trace=trace)
    outs = np.stack([res.results[b]["outT"].T for b in range(B)], axis=0).astype(np.float32)
    return outs, res


def kernel(**inputs):
    inputs = {k: np.asarray(v) for k, v in inputs.items()}
    outs, _ = _run(inputs, trace=False)
    return outs
